# revision 1
# baseline (speedup 1.0000x reference)
"""Trainium2 Bass kernel for LoRACrossAttnProcessor.

Strategy:
- Host: fold LoRA (W_eff = W + up @ down, exact), pre-transpose X/E/W so all
  device matmuls contract over the partition dim with no on-chip transposes.
- Shard: data-parallel over batch, 2 batch items per core, 8 cores.
- Device (per core, all fp32r = fp32 rounded to 11-bit mantissa; fp32 PSUM):
    K.T = Wk_eff @ E.T   [1280, 154]   (both batches, N padded to 256)
    V   = E @ Wv_eff.T   [77, 1280]    (per batch, natural layout)
    Q.T = Wq_eff @ X.T   [1280, 1024]  (per batch)
    per (batch, head): scores.T = (K.T)_h.T-chunks @ (Q.T)_h  -> [77, 1024]
      exps = exp(scores.T * scale)  (ACT, fused scale)
      sumexp = ones.T @ exps (PE), recip (DVE), partition-broadcast (DMA)
      A.T_h = V_h.T @ exps via col-tiled matmuls, normalized by recip (DVE STT)
    O = A @ Wo_eff.T  [1024, 1280]  (natural layout, streamed out)
- Host: gather batches, add bo.
"""

import numpy as np
from contextlib import ExitStack

import concourse.bass as bass
import concourse.mybir as mybir
import concourse.tile as tile
from concourse import bacc
from concourse.bass_utils import run_bass_kernel_spmd

F32 = mybir.dt.float32
F32R = mybir.dt.float32r
AF = mybir.ActivationFunctionType
MULT = mybir.AluOpType.mult

H = 8
B, S, C = 16, 1024, 1280
SENC, CENC = 77, 1024
D = C // H  # 160
NCORES = 8
BPC = B // NCORES  # 2 batches per core
P = 128
NCI_Q = C // P  # 10 contraction tiles for Q/O proj
NCI_KV = CENC // P  # 8 contraction tiles for K/V proj
NCO = C // P  # 10 output-channel tiles
NST = S // 512  # 2 seq chunks of 512
EPAD = 256  # padded encoder column count (2*77 -> 256)
ATTN_SCALE = 1.0 / float(np.sqrt(D))
OCHUNKS = [(0, 512), (512, 512), (1024, 256)]


def head_chunks(h):
    """Split head h's channel range [160h, 160h+160) into PE-tile-aligned
    blocks: size in {32, 64, 128}, offset % size == 0 within a 128-tile.

    Returns [(tile, offset, size, local_d0)]."""
    out = []
    g0, g1 = D * h, D * (h + 1)
    g = g0
    while g < g1:
        t, off = divmod(g, P)
        rem = min(P - off, g1 - g)
        s = 128
        while s > rem or off % s != 0:
            s //= 2
        assert s >= 32
        out.append((t, off, s, g - g0))
        g += s
    return out


def aligned_ranges(r0, r1):
    """Decompose [r0, r1) (within one 128 tile) into blocks of size 32/64/128
    with offset % size == 0 (SBUF partition-access alignment rule)."""
    out = []
    g = r0
    while g < r1:
        s = 128
        while s > r1 - g or g % s != 0:
            s //= 2
        out.append((g, s))
        g += s
    return out


def build():
    nc = bacc.Bacc("TRN2", target_bir_lowering=False, debug=False)
    xt_d = nc.dram_tensor("xt", [BPC, C, S], F32, kind="ExternalInput")
    et_d = nc.dram_tensor("et", [CENC, EPAD], F32, kind="ExternalInput")
    wqt_d = nc.dram_tensor("wqt", [C, C], F32, kind="ExternalInput")
    wkt_d = nc.dram_tensor("wkt", [CENC, C], F32, kind="ExternalInput")
    wvt_d = nc.dram_tensor("wvt", [CENC, C], F32, kind="ExternalInput")
    wot_d = nc.dram_tensor("wot", [C, C], F32, kind="ExternalInput")
    out_d = nc.dram_tensor("out", [BPC, S, C], F32, kind="ExternalOutput")

    with tile.TileContext(nc) as tc, ExitStack() as ctx:
        big = ctx.enter_context(tc.tile_pool(name="big", bufs=3))
        wblk = ctx.enter_context(tc.tile_pool(name="wblk", bufs=2))
        raw = ctx.enter_context(tc.tile_pool(name="raw", bufs=2))
        persist = ctx.enter_context(tc.tile_pool(name="persist", bufs=1))
        expp = ctx.enter_context(tc.tile_pool(name="expp", bufs=2))
        smallp = ctx.enter_context(tc.tile_pool(name="smallp", bufs=2))
        stag = ctx.enter_context(tc.tile_pool(name="stag", bufs=2))
        psum = ctx.enter_context(tc.tile_pool(name="psum", bufs=7, space="PSUM"))

        rnd_engines = [nc.vector, nc.vector]

        # ---- constants ----
        ones77f = persist.tile([SENC, 1], F32, tag="ones77f")
        nc.vector.memset(ones77f, 1.0)
        ones77r = persist.tile([SENC, 1], F32R, tag="ones77r")
        nc.vector.tensor_copy(out=ones77r, in_=ones77f)
        zeros_f = persist.tile([P, 2 * SENC], F32, tag="zeros_f")
        nc.vector.memset(zeros_f, 0.0)

        # ---- load & round E.T  [1024, 256] -> et_r [128, 8, 256] ----
        et_r = persist.tile([P, NCI_KV, EPAD], F32R, tag="et")
        for ci in range(NCI_KV):
            rw = raw.tile([P, NCI_Q, P], F32, tag="raw")
            nc.sync.dma_start(
                out=rw[:, :2, :].rearrange("p a b -> p (a b)"),
                in_=et_d.ap()[ci * P : (ci + 1) * P, :],
            )
            rnd_engines[ci % 2].tensor_copy(
                out=et_r[:, ci, :], in_=rw[:, :2, :].rearrange("p a b -> p (a b)")
            )

        # ---- K.T projection (both batches): kt_r[t] = [128, 154] ----
        kt_r = []
        for t in range(NCO):
            blk = wblk.tile([P, NCI_Q, EPAD], F32R, tag="wblk")
            for ci in range(NCI_KV):
                rw = raw.tile([P, NCI_Q, P], F32, tag="raw")
                nc.sync.dma_start(
                    out=rw[:, 0, :],
                    in_=wkt_d.ap()[ci * P : (ci + 1) * P, t * P : (t + 1) * P],
                )
                rnd_engines[ci % 2].tensor_copy(
                    out=blk[:, ci, :P], in_=rw[:, 0, :]
                )
            ps = psum.tile([P, EPAD], F32, tag="ps")
            for ci in range(NCI_KV):
                nc.tensor.matmul(
                    ps,
                    blk[:, ci, :P],
                    et_r[:, ci, :],
                    start=(ci == 0),
                    stop=(ci == NCI_KV - 1),
                )
            # Two parity-masked K.T copies: even heads' rows in kte (odd rows
            # zero) and vice versa. Scores matmuls can then use full 128-row
            # base-0 tiles; zeros kill the other heads' contributions.
            # (Accumulating matmuls from different PE row-groups into one
            # PSUM crash at runtime, so per-head row-chunks are not usable.)
            kte = persist.tile([P, 2 * SENC], F32R, tag=f"kte{t}", name=f"kte{t}")
            kto = persist.tile([P, 2 * SENC], F32R, tag=f"kto{t}", name=f"kto{t}")
            nc.vector.tensor_copy(out=kte, in_=zeros_f[:, : 2 * SENC])
            nc.vector.tensor_copy(out=kto, in_=zeros_f[:, : 2 * SENC])
            for h in range(H):
                r0 = max(D * h, P * t)
                r1 = min(D * h + D, P * t + P)
                if r0 >= r1:
                    continue
                dst = kte if h % 2 == 0 else kto
                for o, s in aligned_ranges(r0 - P * t, r1 - P * t):
                    nc.vector.tensor_copy(
                        out=dst[o : o + s, :], in_=ps[o : o + s, : 2 * SENC]
                    )
            kt_r.append((kte, kto))

        # ---- V projection (per batch, natural layout): v_nat[b] [77, 1280] ----
        v_nat = []
        for b in range(BPC):
            v_nat.append(
                persist.tile([SENC, C], F32R, tag=f"vnat{b}", name=f"vnat{b}")
            )
        for cc in range(0, C, 256):
            blk = wblk.tile([P, NCI_Q, EPAD], F32R, tag="wblk")
            for ci in range(NCI_KV):
                rw = raw.tile([P, NCI_Q, P], F32, tag="raw")
                nc.sync.dma_start(
                    out=rw[:, :2, :].rearrange("p a b -> p (a b)"),
                    in_=wvt_d.ap()[ci * P : (ci + 1) * P, cc : cc + 256],
                )
                rnd_engines[ci % 2].tensor_copy(
                    out=blk[:, ci, :],
                    in_=rw[:, :2, :].rearrange("p a b -> p (a b)"),
                )
            for b in range(BPC):
                ps = psum.tile([SENC, 512], F32, tag="ps")
                for ci in range(NCI_KV):
                    nc.tensor.matmul(
                        ps[:, :256],
                        et_r[:, ci, b * SENC : (b + 1) * SENC],
                        blk[:, ci, :],
                        start=(ci == 0),
                        stop=(ci == NCI_KV - 1),
                    )
                nc.vector.tensor_copy(
                    out=v_nat[b][:, cc : cc + 256], in_=ps[:, :256]
                )

        # ---- load & round X.T per batch: xt_r[b] [128, 10, 1024] ----
        xt_r = [None] * BPC
        for b in range(BPC):
            xt_r[b] = big.tile([P, NCI_Q, S], F32R, tag="big", name=f"xt{b}")
            for ci in range(NCI_Q):
                rw = raw.tile([P, NCI_Q, P], F32, tag="raw")
                nc.sync.dma_start(
                    out=rw[:, :8, :].rearrange("p a b -> p (a b)"),
                    in_=xt_d.ap()[b, ci * P : (ci + 1) * P, :],
                )
                rnd_engines[ci % 2].tensor_copy(
                    out=xt_r[b][:, ci, :],
                    in_=rw[:, :8, :].rearrange("p a b -> p (a b)"),
                )

        # ---- Q.T projection, batch-major (Wq streamed per batch) ----
        qt_r = [None] * BPC
        for b in range(BPC):
            qt_r[b] = big.tile([P, NCO, S], F32R, tag="big", name=f"qt{b}")
            for co in range(NCO):
                blk = wblk.tile([P, NCI_Q, EPAD], F32R, tag="wblk")
                rwb = raw.tile([P, NCI_Q, P], F32, tag="raw")
                nc.sync.dma_start(
                    out=rwb,
                    in_=wqt_d.ap()[:, co * P : (co + 1) * P].rearrange(
                        "(ci p) c -> p ci c", p=P
                    ),
                )
                for ci in range(NCI_Q):
                    rnd_engines[ci % 2].tensor_copy(
                        out=blk[:, ci, :P], in_=rwb[:, ci, :]
                    )
                for st in range(NST):
                    ps = psum.tile([P, 512], F32, tag="ps")
                    for ci in range(NCI_Q):
                        nc.tensor.matmul(
                            ps,
                            blk[:, ci, :P],
                            xt_r[b][:, ci, st * 512 : st * 512 + 512],
                            start=(ci == 0),
                            stop=(ci == NCI_Q - 1),
                        )
                    nc.vector.tensor_copy(
                        out=qt_r[b][:, co, st * 512 : st * 512 + 512], in_=ps
                    )

        # ---- attention per (batch, head) -> at_r[b] [128, 10, 1024] ----
        at_r = [None] * BPC
        for b in range(BPC):
            at_r[b] = big.tile([P, NCO, S], F32R, tag="big", name=f"at{b}")
            for h in range(H):
                hch = head_chunks(h)
                for st in range(NST):
                    sl = slice(st * 512, st * 512 + 512)
                    # scores.T [77, 512]: full 128-row tiles of parity-masked
                    # K.T accumulated over the tiles this head touches.
                    tiles = sorted({t for (t, _, _, _) in hch})
                    ps_s = psum.tile([SENC, 512], F32, tag="ps")
                    for i, t in enumerate(tiles):
                        nc.tensor.matmul(
                            ps_s,
                            kt_r[t][h % 2][:, b * SENC : (b + 1) * SENC],
                            qt_r[b][:, t, sl],
                            start=(i == 0),
                            stop=(i == len(tiles) - 1),
                        )
                    exps = expp.tile([SENC, 512], F32R, tag="exps")
                    nc.scalar.activation(
                        out=exps, in_=ps_s, func=AF.Exp, scale=ATTN_SCALE
                    )
                    # sumexp [1, 512] on PE; reciprocal; partition-broadcast
                    ps_se = psum.tile([1, 512], F32, tag="ps")
                    nc.tensor.matmul(ps_se, ones77r, exps, start=True, stop=True)
                    rec = smallp.tile([1, 512], F32, tag="rec")
                    nc.vector.reciprocal(out=rec, in_=ps_se)
                    bc = smallp.tile([P, 512], F32, tag="bc")
                    nc.gpsimd.partition_broadcast(bc, rec)
                    # A.T_h = V_h.T @ exps, landed at global partition offsets
                    # via col-tiling; normalize by bc while copying to SBUF.
                    for t, off, size, l0 in hch:
                        ps_av = psum.tile([P, 512], F32, tag="ps")
                        nc.tensor.matmul(
                            ps_av[0:size, :],
                            v_nat[b][:, D * h + l0 : D * h + l0 + size],
                            exps,
                            start=True,
                            stop=True,
                        )
                        avt = smallp.tile([P, 512], F32R, tag="avt")
                        nc.vector.scalar_tensor_tensor(
                            out=avt[0:size, :],
                            in0=ps_av[0:size, :],
                            scalar=1.0,
                            in1=bc[0:size, :],
                            op0=MULT,
                            op1=MULT,
                        )
                        # fp32r matmuls can't target PSUM partition offsets;
                        # DMA does the partition shift into the assembled A.T.
                        nc.sync.dma_start(
                            out=at_r[b][off : off + size, t, sl],
                            in_=avt[0:size, :],
                        )

        # ---- O projection as O.T (Wo streamed once, stage-major) ----
        # O.T[co, m] = sum_ch Wo_eff[co, ch] A[m, ch]; DMA writes DRAM with a
        # transposed access pattern (partition dim -> channel, 4B stride).
        for co in range(NCO):
            blk = wblk.tile([P, NCI_Q, EPAD], F32R, tag="wblk")
            rwb = raw.tile([P, NCI_Q, P], F32, tag="raw")
            nc.sync.dma_start(
                out=rwb,
                in_=wot_d.ap()[:, co * P : (co + 1) * P].rearrange(
                    "(ci p) c -> p ci c", p=P
                ),
            )
            for ci in range(NCI_Q):
                rnd_engines[ci % 2].tensor_copy(
                    out=blk[:, ci, :P], in_=rwb[:, ci, :]
                )
            for b in range(BPC):
                for st in range(NST):
                    ps = psum.tile([P, 512], F32, tag="ps")
                    for ci in range(NCI_Q):
                        nc.tensor.matmul(
                            ps,
                            blk[:, ci, :P],
                            at_r[b][:, ci, st * 512 : st * 512 + 512],
                            start=(ci == 0),
                            stop=(ci == NCI_Q - 1),
                        )
                    ot = stag.tile([P, 512], F32, tag="ot")
                    nc.scalar.copy(out=ot, in_=ps)
                    nc.sync.dma_start(
                        out=out_d.ap()[
                            b, st * 512 : st * 512 + 512, co * P : (co + 1) * P
                        ].rearrange("s c -> c s"),
                        in_=ot,
                    )

    nc.compile()
    return nc


_NC_CACHE = []


def _get_nc():
    if not _NC_CACHE:
        _NC_CACHE.append(build())
    return _NC_CACHE[0]


def make_in_maps(hidden_states, encoder_hidden_states, Wq, Wk, Wv, Wo,
                 q_down, q_up, k_down, k_up, v_down, v_up, o_down, o_up):
    wq = (Wq.astype(np.float64) + q_up.astype(np.float64) @ q_down.astype(np.float64))
    wk = (Wk.astype(np.float64) + k_up.astype(np.float64) @ k_down.astype(np.float64))
    wv = (Wv.astype(np.float64) + v_up.astype(np.float64) @ v_down.astype(np.float64))
    wo = (Wo.astype(np.float64) + o_up.astype(np.float64) @ o_down.astype(np.float64))
    wqt = np.ascontiguousarray(wq.T.astype(np.float32))
    wkt = np.ascontiguousarray(wk.T.astype(np.float32))
    wvt = np.ascontiguousarray(wv.T.astype(np.float32))
    wot = np.ascontiguousarray(wo.T.astype(np.float32))

    in_maps = []
    for c in range(NCORES):
        hs = hidden_states[c * BPC : (c + 1) * BPC]  # [2, S, C]
        xt = np.ascontiguousarray(hs.transpose(0, 2, 1).astype(np.float32))
        enc = encoder_hidden_states[c * BPC : (c + 1) * BPC]  # [2, 77, 1024]
        et = np.zeros((CENC, EPAD), np.float32)
        for b in range(BPC):
            et[:, b * SENC : (b + 1) * SENC] = enc[b].T
        in_maps.append(
            {"xt": xt, "et": et, "wqt": wqt, "wkt": wkt, "wvt": wvt, "wot": wot}
        )
    return in_maps


def kernel(hidden_states, encoder_hidden_states, Wq, Wk, Wv, Wo, bo,
           q_down, q_up, k_down, k_up, v_down, v_up, o_down, o_up):
    nc = _get_nc()
    in_maps = make_in_maps(
        hidden_states, encoder_hidden_states, Wq, Wk, Wv, Wo,
        q_down, q_up, k_down, k_up, v_down, v_up, o_down, o_up,
    )
    res = run_bass_kernel_spmd(nc, in_maps, list(range(NCORES)))
    out = np.concatenate([res.results[c]["out"] for c in range(NCORES)], axis=0)
    out = out + bo.astype(np.float32)[None, None, :]
    return out.astype(np.float32)



# revision 4
# speedup vs baseline: 24.0741x; 24.0741x over previous
"""Trainium2 Bass kernel for LoRACrossAttnProcessor (v2, bf16).

Strategy:
- Host: fold LoRA (W_eff = W + up @ down, exact in f64), permute the qkv
  output channels (rows of Wq/Wk/Wv, cols of Wo) so each head owns one full
  128-row tile (tiles 0-7) plus a 32-row slice of the shared leftover tiles
  (8-9).  Pre-transpose and cast everything to bf16 on the host.
- Shard: data-parallel over batch, 2 batch items per core, 8 cores.
- Device (per core, bf16 matmuls, fp32 PSUM):
    K.T tiles  = Wk_p @ E.T      [128, 154] per tile (both batches at once)
    V          = E @ Wv_p.T      [77, 1280] per batch
    Q.T        = Wq_p @ X.T      [128, 10, 1024] bf16 per batch
    per (batch, st-chunk, head):
      scores.T = kt[h].T-slice @ Q.T chunks      [77, 512] fp32 PSUM
      exps     = exp(scores.T * scale) -> bf16   (ACT, fused scale)
      sumexp   = ones.T @ exps (PE), recip (DVE), partition-broadcast (Pool)
      A.T main = V[:, head-tile] MM -> STT normalize -> at[:, h, :] (no DMA)
      A.T left = V[:, leftover]  MM -> STT -> 32-row SBUF-shift DMA
    O.T        = Wo_p @ A.T     [128, 1024] f32 staged, contiguous DMA out
- Host: transpose O.T -> O, add bo.
"""

import numpy as np
import ml_dtypes
from contextlib import ExitStack

import concourse.bass as bass
import concourse.mybir as mybir
import concourse.tile as tile
from concourse import bacc
from concourse.bass_utils import run_bass_kernel_spmd

F32 = mybir.dt.float32
BF16 = mybir.dt.bfloat16
AF = mybir.ActivationFunctionType
MULT = mybir.AluOpType.mult

H = 8
B, S, C = 16, 1024, 1280
SENC, CENC = 77, 1024
D = C // H  # 160
NCORES = 8
BPC = B // NCORES  # 2
P = 128
NCI_Q = C // P  # 10
NCI_KV = CENC // P  # 8
NCO = C // P  # 10
EW = 2 * SENC  # 154, both batches' encoder tokens side by side
ATTN_SCALE = 1.0 / float(np.sqrt(D))
ST = (slice(0, 512), slice(512, 1024))


def head_perm():
    """New channel order: head h gets rows [128h,128h+128) (its first 128
    dims) and rows [1024+32h, 1024+32h+32) (its last 32 dims)."""
    perm = []
    for h in range(H):
        perm.extend(range(D * h, D * h + P))
    for h in range(H):
        perm.extend(range(D * h + P, D * h + D))
    return np.asarray(perm)


def build():
    nc = bacc.Bacc("TRN2", target_bir_lowering=False, debug=False)
    xt_d = nc.dram_tensor("xt", [BPC, NCI_Q, P, S], BF16, kind="ExternalInput")
    et_d = nc.dram_tensor("et", [NCI_KV, P, EW], BF16, kind="ExternalInput")
    wk_d = nc.dram_tensor("wk", [NCI_KV, P, C], BF16, kind="ExternalInput")
    wv_d = nc.dram_tensor("wv", [NCI_KV, P, C], BF16, kind="ExternalInput")
    wq_d = nc.dram_tensor("wq", [NCI_Q, P, C], BF16, kind="ExternalInput")
    wo_d = nc.dram_tensor("wo", [NCI_Q, P, C], BF16, kind="ExternalInput")
    otd_d = nc.dram_tensor("otd", [BPC, C, S], F32, kind="ExternalOutput")

    with tile.TileContext(nc) as tc, ExitStack() as ctx:
        wpool = ctx.enter_context(tc.tile_pool(name="wpool", bufs=2))
        apool = ctx.enter_context(tc.tile_pool(name="apool", bufs=4))
        persist = ctx.enter_context(tc.tile_pool(name="persist", bufs=1))
        expp = ctx.enter_context(tc.tile_pool(name="expp", bufs=3))
        bcp = ctx.enter_context(tc.tile_pool(name="bcp", bufs=3))
        recp = ctx.enter_context(tc.tile_pool(name="recp", bufs=3))
        lost = ctx.enter_context(tc.tile_pool(name="lost", bufs=2))
        ostg = ctx.enter_context(tc.tile_pool(name="ostg", bufs=2))
        psmm = ctx.enter_context(tc.tile_pool(name="psmm", bufs=4, space="PSUM"))
        psse = ctx.enter_context(tc.tile_pool(name="psse", bufs=2, space="PSUM"))
        pslo = ctx.enter_context(tc.tile_pool(name="pslo", bufs=2, space="PSUM"))

        # ---- constants & persistent buffers ----
        ones77 = persist.tile([SENC, 1], BF16, tag="ones77")
        nc.vector.memset(ones77, 1.0)
        # kt[t]: K.T rows of head-tile t (pure single head, t=0..7)
        kt = [persist.tile([P, EW], BF16, tag=f"kt{t}", name=f"kt{t}") for t in range(H)]
        # ktm[i][m]: leftover tile 8+i with only head (4i+m)'s 32 rows nonzero
        ktm = [
            [
                persist.tile(
                    [P, EW], BF16, tag=f"ktm{i}_{m}", name=f"ktm{i}_{m}"
                )
                for m in range(4)
            ]
            for i in range(2)
        ]
        for i in range(2):
            for m in range(4):
                nc.vector.memset(ktm[i][m], 0.0)
        v_nat = [
            persist.tile([SENC, C], BF16, tag=f"vnat{b}", name=f"vnat{b}")
            for b in range(BPC)
        ]
        et = persist.tile([P, NCI_KV, EW], BF16, tag="et")

        # ---- input DMAs (issued early; tile framework orders by deps) ----
        nc.scalar.dma_start(out=et, in_=et_d.ap().rearrange("ci p c -> p ci c"))
        wk = wpool.tile([P, NCI_KV, C], BF16, tag="w", name="wk")
        nc.sync.dma_start(out=wk, in_=wk_d.ap().rearrange("ci p c -> p ci c"))
        wv = wpool.tile([P, NCI_KV, C], BF16, tag="w", name="wv")
        nc.scalar.dma_start(out=wv, in_=wv_d.ap().rearrange("ci p c -> p ci c"))
        xt = []
        for b in range(BPC):
            x = apool.tile([P, NCI_Q, S], BF16, tag="act", name=f"xt{b}")
            eng = nc.sync if b == 0 else nc.scalar
            eng.dma_start(
                out=x, in_=xt_d.ap()[b].rearrange("ci p c -> p ci c")
            )
            xt.append(x)

        # ---- K.T projection: both batches at once ----
        for t in range(NCO):
            ps = psmm.tile([P, EW], F32, tag="mm")
            for ci in range(NCI_KV):
                nc.tensor.matmul(
                    ps,
                    wk[:, ci, t * P : (t + 1) * P],
                    et[:, ci, :],
                    start=(ci == 0),
                    stop=(ci == NCI_KV - 1),
                )
            if t < H:
                nc.vector.tensor_copy(out=kt[t], in_=ps)
            else:
                for m in range(4):
                    nc.vector.tensor_copy(
                        out=ktm[t - H][m][32 * m : 32 * m + 32, :],
                        in_=ps[32 * m : 32 * m + 32, :],
                    )

        # ---- V projection (per batch, natural layout) ----
        VCH = [(0, 512), (512, 512), (1024, 256)]
        for b in range(BPC):
            pss = [psmm.tile([SENC, 512], F32, tag="mm", name=f"psv{b}_{j}") for j in range(3)]
            for ci in range(NCI_KV):
                for j, (cc, w) in enumerate(VCH):
                    nc.tensor.matmul(
                        pss[j][:, :w],
                        et[:, ci, b * SENC : (b + 1) * SENC],
                        wv[:, ci, cc : cc + w],
                        start=(ci == 0),
                        stop=(ci == NCI_KV - 1),
                    )
            for j, (cc, w) in enumerate(VCH):
                nc.vector.tensor_copy(
                    out=v_nat[b][:, cc : cc + w], in_=pss[j][:, :w]
                )

        # wq reuses wk's slot (waits for K proj), wo reuses wv's slot
        wq = wpool.tile([P, NCI_Q, C], BF16, tag="w", name="wq")
        nc.sync.dma_start(out=wq, in_=wq_d.ap().rearrange("ci p c -> p ci c"))
        wo = wpool.tile([P, NCI_Q, C], BF16, tag="w", name="wo")
        nc.scalar.dma_start(out=wo, in_=wo_d.ap().rearrange("ci p c -> p ci c"))

        def cpy_st(st, out, in_):
            if st == 0:
                nc.vector.tensor_copy(out=out, in_=in_)
            else:
                nc.scalar.copy(out=out, in_=in_)

        def q_proj(b, qt):
            for co in range(NCO):
                ps = [
                    psmm.tile([P, 512], F32, tag="mm", name=f"psq{co}_{st}")
                    for st in range(2)
                ]
                for ci in range(NCI_Q):
                    for st in range(2):
                        nc.tensor.matmul(
                            ps[st],
                            wq[:, ci, co * P : (co + 1) * P],
                            xt[b][:, ci, ST[st]],
                            start=(ci == 0),
                            stop=(ci == NCI_Q - 1),
                        )
                for st in range(2):
                    cpy_st(st, qt[:, co, ST[st]], ps[st])

        def attn_head(b, qt, at, st, h):
            """Emit scores for head h; returns closures for the tail."""
            sl = ST[st]
            ps_s = psmm.tile([SENC, 512], F32, tag="mm")
            i, m = divmod(h, 4)
            nc.tensor.matmul(
                ps_s, kt[h][:, b * SENC : (b + 1) * SENC], qt[:, h, sl],
                start=True, stop=False,
            )
            nc.tensor.matmul(
                ps_s, ktm[i][m][:, b * SENC : (b + 1) * SENC],
                qt[:, H + i, sl], start=False, stop=True,
            )
            return ps_s

        def attn_tail(b, qt, at, st, h, ps_s):
            sl = ST[st]
            exps = expp.tile([SENC, 512], BF16, tag="exps")
            nc.scalar.activation(out=exps, in_=ps_s, func=AF.Exp, scale=ATTN_SCALE)
            ps_se = psse.tile([1, 512], F32, tag="se")
            nc.tensor.matmul(ps_se, ones77, exps, start=True, stop=True)
            rec = recp.tile([1, 512], F32, tag="rec")
            nc.vector.reciprocal(out=rec, in_=ps_se)
            bc = bcp.tile([P, 512], F32, tag="bc")
            nc.gpsimd.partition_broadcast(bc, rec)
            ps_av = psmm.tile([P, 512], F32, tag="mm")
            nc.tensor.matmul(
                ps_av, v_nat[b][:, P * h : P * (h + 1)], exps,
                start=True, stop=True,
            )
            nc.vector.scalar_tensor_tensor(
                out=at[:, h, sl], in0=ps_av, scalar=1.0, in1=bc,
                op0=MULT, op1=MULT,
            )
            ps_lo = pslo.tile([32, 512], F32, tag="lo")
            nc.tensor.matmul(
                ps_lo, v_nat[b][:, 1024 + 32 * h : 1024 + 32 * h + 32], exps,
                start=True, stop=True,
            )
            lo = lost.tile([32, 512], BF16, tag="lo")
            nc.vector.scalar_tensor_tensor(
                out=lo, in0=ps_lo, scalar=1.0, in1=bc[0:32, :],
                op0=MULT, op1=MULT,
            )
            i, m = divmod(h, 4)
            nc.sync.dma_start(
                out=at[32 * m : 32 * m + 32, H + i, sl], in_=lo
            )

        def attn(b, qt, at):
            # software-pipelined: scores of (st,h) are emitted one step ahead
            # of the tail of the previous head so the PE never head-of-line
            # blocks on the ACT exp round trip.
            items = [(st, h) for st in range(2) for h in range(H)]
            pend = None
            for st, h in items:
                ps_s = attn_head(b, qt, at, st, h)
                if pend is not None:
                    attn_tail(b, qt, at, *pend)
                pend = (st, h, ps_s)
            attn_tail(b, qt, at, *pend)

        def o_proj(b, at):
            for co in range(NCO):
                ost = ostg.tile([P, S], F32, tag="ost")
                ps = [
                    psmm.tile([P, 512], F32, tag="mm", name=f"pso{co}_{st}")
                    for st in range(2)
                ]
                for ci in range(NCI_Q):
                    for st in range(2):
                        nc.tensor.matmul(
                            ps[st],
                            wo[:, ci, co * P : (co + 1) * P],
                            at[:, ci, ST[st]],
                            start=(ci == 0),
                            stop=(ci == NCI_Q - 1),
                        )
                for st in range(2):
                    cpy_st(st, ost[:, ST[st]], ps[st])
                nc.sync.dma_start(
                    out=otd_d.ap()[b, co * P : (co + 1) * P, :], in_=ost
                )

        qt0 = apool.tile([P, NCO, S], BF16, tag="act", name="qt0")
        q_proj(0, qt0)
        at0 = apool.tile([P, NCI_Q, S], BF16, tag="act", name="at0")
        attn(0, qt0, at0)
        qt1 = apool.tile([P, NCO, S], BF16, tag="act", name="qt1")
        q_proj(1, qt1)
        at1 = apool.tile([P, NCI_Q, S], BF16, tag="act", name="at1")
        attn(1, qt1, at1)
        o_proj(0, at0)
        o_proj(1, at1)

    nc.compile()
    return nc


_NC_CACHE = []


def _get_nc():
    if not _NC_CACHE:
        _NC_CACHE.append(build())
    return _NC_CACHE[0]


def make_in_maps(hidden_states, encoder_hidden_states, Wq, Wk, Wv, Wo,
                 q_down, q_up, k_down, k_up, v_down, v_up, o_down, o_up):
    f64 = np.float64
    wq = Wq.astype(f64) + q_up.astype(f64) @ q_down.astype(f64)
    wk = Wk.astype(f64) + k_up.astype(f64) @ k_down.astype(f64)
    wv = Wv.astype(f64) + v_up.astype(f64) @ v_down.astype(f64)
    wo = Wo.astype(f64) + o_up.astype(f64) @ o_down.astype(f64)
    perm = head_perm()
    bf = ml_dtypes.bfloat16
    # stationary layouts: w*[ci, p, c] = W_p[c, ci*128+p]  (= W_p.T reshaped)
    wq_h = np.ascontiguousarray(
        wq[perm, :].T.reshape(NCI_Q, P, C).astype(bf)
    )
    wk_h = np.ascontiguousarray(
        wk[perm, :].T.reshape(NCI_KV, P, C).astype(bf)
    )
    wv_h = np.ascontiguousarray(
        wv[perm, :].T.reshape(NCI_KV, P, C).astype(bf)
    )
    wo_h = np.ascontiguousarray(
        wo[:, perm].T.reshape(NCI_Q, P, C).astype(bf)
    )

    in_maps = []
    for c in range(NCORES):
        hs = hidden_states[c * BPC : (c + 1) * BPC]  # [2, S, C]
        xt = np.ascontiguousarray(
            hs.transpose(0, 2, 1).reshape(BPC, NCI_Q, P, S).astype(bf)
        )
        enc = encoder_hidden_states[c * BPC : (c + 1) * BPC]  # [2, 77, 1024]
        et = np.zeros((CENC, EW), np.float32)
        for b in range(BPC):
            et[:, b * SENC : (b + 1) * SENC] = enc[b].T
        et = np.ascontiguousarray(et.reshape(NCI_KV, P, EW).astype(bf))
        in_maps.append(
            {"xt": xt, "et": et, "wq": wq_h, "wk": wk_h, "wv": wv_h,
             "wo": wo_h}
        )
    return in_maps


def kernel(hidden_states, encoder_hidden_states, Wq, Wk, Wv, Wo, bo,
           q_down, q_up, k_down, k_up, v_down, v_up, o_down, o_up):
    nc = _get_nc()
    in_maps = make_in_maps(
        hidden_states, encoder_hidden_states, Wq, Wk, Wv, Wo,
        q_down, q_up, k_down, k_up, v_down, v_up, o_down, o_up,
    )
    res = run_bass_kernel_spmd(nc, in_maps, list(range(NCORES)))
    out = np.empty((B, S, C), np.float32)
    for c in range(NCORES):
        ot = res.results[c]["otd"]  # [BPC, C, S]
        for b in range(BPC):
            out[c * BPC + b] = ot[b].T
    out += bo.astype(np.float32)[None, None, :]
    return out


# revision 5
# speedup vs baseline: 24.4723x; 1.0165x over previous
"""Trainium2 Bass kernel for LoRACrossAttnProcessor (v3, bf16, interleaved).

Strategy:
- Host: fold LoRA (W_eff = W + up @ down, exact in f64), permute the qkv
  output channels (rows of Wq/Wk/Wv, cols of Wo) so each head owns one full
  128-row tile (tiles 0-7) plus a 32-row slice of the shared leftover tiles
  (8-9).  Pre-transpose and cast everything to bf16 on the host.
- Shard: data-parallel over batch, 2 batch items per core, 8 cores.
- Device (per core, bf16 matmuls, fp32 PSUM):
    K.T tiles  = Wk_p @ E.T      [128, 154] per tile (both batches at once)
    V          = E @ Wv_p.T      [77, 1024] main + [77, 8, 33] leftover+ones
    Q.T        = Wq_p @ X.T      [128, 10, 1024] bf16 per batch
    per (batch, head):  (one unit covers both 512-col seq chunks)
      scores.T = kt/ktm MMs -> [77, 1024] fp32 PSUM (4 MMs)
      exps     = exp(scores.T * scale) -> bf16 [77, 1024]  (one ACT op)
      A.T main = V[:, head-tile] @ exps          [128, 512] x2
      A.T left = [V_left | 1] @ exps -> [33, 512] x2; row 32 = sumexp
      recip (DVE), partition-broadcast (Pool), STT-normalize -> at
    O.T        = Wo_p @ A.T     [128, 1024] f32 staged, contiguous DMA out
- Emission interleaves Q proj of batch 1 into attention of batch 0 (and
  O proj of batch 0 into attention of batch 1) so the PE stays busy while
  DVE/ACT/Pool drain the softmax tails.
- Host: transpose O.T -> O, add bo.
"""

import numpy as np
import ml_dtypes
from contextlib import ExitStack

import concourse.bass as bass
import concourse.mybir as mybir
import concourse.tile as tile
from concourse import bacc
from concourse.bass_utils import run_bass_kernel_spmd

F32 = mybir.dt.float32
BF16 = mybir.dt.bfloat16
AF = mybir.ActivationFunctionType
MULT = mybir.AluOpType.mult

H = 8
B, S, C = 16, 1024, 1280
SENC, CENC = 77, 1024
D = C // H  # 160
NCORES = 8
BPC = B // NCORES  # 2
P = 128
NCI_Q = C // P  # 10
NCI_KV = CENC // P  # 8
NCO = C // P  # 10
EW = 2 * SENC  # 154, both batches' encoder tokens side by side
ATTN_SCALE = 1.0 / float(np.sqrt(D))
ST = (slice(0, 512), slice(512, 1024))


def head_perm():
    """New channel order: head h gets rows [128h,128h+128) (its first 128
    dims) and rows [1024+32h, 1024+32h+32) (its last 32 dims)."""
    perm = []
    for h in range(H):
        perm.extend(range(D * h, D * h + P))
    for h in range(H):
        perm.extend(range(D * h + P, D * h + D))
    return np.asarray(perm)


def build():
    nc = bacc.Bacc("TRN2", target_bir_lowering=False, debug=False)
    xt_d = nc.dram_tensor("xt", [BPC, NCI_Q, P, S], BF16, kind="ExternalInput")
    et_d = nc.dram_tensor("et", [NCI_KV, P, EW], BF16, kind="ExternalInput")
    wk_d = nc.dram_tensor("wk", [NCI_KV, P, C], BF16, kind="ExternalInput")
    wv_d = nc.dram_tensor("wv", [NCI_KV, P, C], BF16, kind="ExternalInput")
    wq_d = nc.dram_tensor("wq", [NCI_Q, P, C], BF16, kind="ExternalInput")
    wo_d = nc.dram_tensor("wo", [NCI_Q, P, C], BF16, kind="ExternalInput")
    otd_d = nc.dram_tensor("otd", [BPC, C, S], F32, kind="ExternalOutput")

    with tile.TileContext(nc) as tc, ExitStack() as ctx:
        wpool = ctx.enter_context(tc.tile_pool(name="wpool", bufs=3))
        apool = ctx.enter_context(tc.tile_pool(name="apool", bufs=4))
        persist = ctx.enter_context(tc.tile_pool(name="persist", bufs=1))
        expp = ctx.enter_context(tc.tile_pool(name="expp", bufs=2))
        bcp = ctx.enter_context(tc.tile_pool(name="bcp", bufs=2))
        recp = ctx.enter_context(tc.tile_pool(name="recp", bufs=2))
        lost = ctx.enter_context(tc.tile_pool(name="lost", bufs=2))
        ostg = ctx.enter_context(tc.tile_pool(name="ostg", bufs=2))
        psmm = ctx.enter_context(tc.tile_pool(name="psmm", bufs=2, space="PSUM"))
        pssc = ctx.enter_context(tc.tile_pool(name="pssc", bufs=1, space="PSUM"))
        psav = ctx.enter_context(tc.tile_pool(name="psav", bufs=2, space="PSUM"))
        pslo = ctx.enter_context(tc.tile_pool(name="pslo", bufs=2, space="PSUM"))

        # ---- persistent buffers ----
        kt = [
            persist.tile([P, EW], BF16, tag=f"kt{t}", name=f"kt{t}")
            for t in range(H)
        ]
        ktm = [
            [
                persist.tile(
                    [P, EW], BF16, tag=f"ktm{i}_{m}", name=f"ktm{i}_{m}"
                )
                for m in range(4)
            ]
            for i in range(2)
        ]
        for i in range(2):
            for m in range(4):
                nc.vector.memset(ktm[i][m], 0.0)
        v_nat = [
            persist.tile([SENC, CENC], BF16, tag=f"vnat{b}", name=f"vnat{b}")
            for b in range(BPC)
        ]
        # [V_leftover(32) | ones] per head: row 32 of the A.T-leftover matmul
        # output is then the softmax denominator.
        vlo = [
            persist.tile([SENC, H, 33], BF16, tag=f"vlo{b}", name=f"vlo{b}")
            for b in range(BPC)
        ]
        for b in range(BPC):
            nc.vector.memset(vlo[b][:, :, 32:33], 1.0)
        et = persist.tile([P, NCI_KV, EW], BF16, tag="et")

        # ---- input DMAs (ring order: sync: wk,wq,xt0; act: et,wv,xt1,wo) ----
        nc.scalar.dma_start(out=et, in_=et_d.ap().rearrange("ci p c -> p ci c"))
        wk = wpool.tile([P, NCI_KV, C], BF16, tag="w", name="wk")
        nc.sync.dma_start(out=wk, in_=wk_d.ap().rearrange("ci p c -> p ci c"))
        wv = wpool.tile([P, NCI_KV, C], BF16, tag="w", name="wv")
        nc.scalar.dma_start(out=wv, in_=wv_d.ap().rearrange("ci p c -> p ci c"))
        wq = wpool.tile([P, NCI_Q, C], BF16, tag="w", name="wq")
        nc.sync.dma_start(out=wq, in_=wq_d.ap().rearrange("ci p c -> p ci c"))
        xt = []
        for b in range(BPC):
            x = apool.tile([P, NCI_Q, S], BF16, tag="act", name=f"xt{b}")
            eng = nc.sync if b == 0 else nc.scalar
            eng.dma_start(
                out=x, in_=xt_d.ap()[b].rearrange("ci p c -> p ci c")
            )
            xt.append(x)

        # ---- K.T projection: both batches at once ----
        for t in range(NCO):
            ps = psmm.tile([P, EW], F32, tag="mm", name=f"psk{t}")
            for ci in range(NCI_KV):
                nc.tensor.matmul(
                    ps,
                    wk[:, ci, t * P : (t + 1) * P],
                    et[:, ci, :],
                    start=(ci == 0),
                    stop=(ci == NCI_KV - 1),
                )
            if t < H:
                nc.vector.tensor_copy(out=kt[t], in_=ps)
            else:
                for m in range(4):
                    nc.vector.tensor_copy(
                        out=ktm[t - H][m][32 * m : 32 * m + 32, :],
                        in_=ps[32 * m : 32 * m + 32, :],
                    )

        # wo reuses wk's slot: DMA waits for K proj, loads during Q proj b0
        wo = wpool.tile([P, NCI_Q, C], BF16, tag="w", name="wo")
        nc.scalar.dma_start(out=wo, in_=wo_d.ap().rearrange("ci p c -> p ci c"))

        # ---- V projection (per batch): main 1024 cols + leftover into vlo --
        VCH = [(0, 512), (512, 512), (1024, 256)]
        for b in range(BPC):
            for j, (cc, w) in enumerate(VCH):
                ps = psmm.tile([SENC, 512], F32, tag="mm", name=f"psv{b}_{j}")
                for ci in range(NCI_KV):
                    nc.tensor.matmul(
                        ps[:, :w],
                        et[:, ci, b * SENC : (b + 1) * SENC],
                        wv[:, ci, cc : cc + w],
                        start=(ci == 0),
                        stop=(ci == NCI_KV - 1),
                    )
                if j < 2:
                    nc.vector.tensor_copy(
                        out=v_nat[b][:, cc : cc + w], in_=ps[:, :w]
                    )
                else:
                    for h in range(H):
                        nc.vector.tensor_copy(
                            out=vlo[b][:, h, 0:32],
                            in_=ps[:, 32 * h : 32 * h + 32],
                        )

        # ---- unit generators ----
        def q_unit(b, qt, co):
            ps = [
                psmm.tile([P, 512], F32, tag="mm", name=f"psq{b}_{co}_{st}")
                for st in range(2)
            ]
            for ci in range(NCI_Q):
                for st in range(2):
                    nc.tensor.matmul(
                        ps[st],
                        wq[:, ci, co * P : (co + 1) * P],
                        xt[b][:, ci, ST[st]],
                        start=(ci == 0),
                        stop=(ci == NCI_Q - 1),
                    )
            for st in range(2):
                nc.scalar.copy(out=qt[:, co, ST[st]], in_=ps[st])

        def o_unit(b, at, co):
            ost = ostg.tile([P, S], F32, tag="ost", name=f"ost{b}_{co}")
            ps = [
                psmm.tile([P, 512], F32, tag="mm", name=f"pso{b}_{co}_{st}")
                for st in range(2)
            ]
            for ci in range(NCI_Q):
                for st in range(2):
                    nc.tensor.matmul(
                        ps[st],
                        wo[:, ci, co * P : (co + 1) * P],
                        at[:, ci, ST[st]],
                        start=(ci == 0),
                        stop=(ci == NCI_Q - 1),
                    )
            for st in range(2):
                nc.scalar.copy(out=ost[:, ST[st]], in_=ps[st])
            nc.sync.dma_start(
                out=otd_d.ap()[b, co * P : (co + 1) * P, :], in_=ost
            )

        def attn_front(b, qt, h):
            """Scores for head h, both seq chunks -> one [77, 1024] PSUM."""
            ps_s = pssc.tile([SENC, S], F32, tag="sc", name=f"sc{b}_{h}")
            i, m = divmod(h, 4)
            for st in range(2):
                nc.tensor.matmul(
                    ps_s[:, ST[st]],
                    kt[h][:, b * SENC : (b + 1) * SENC],
                    qt[:, h, ST[st]],
                    start=True,
                    stop=False,
                )
                nc.tensor.matmul(
                    ps_s[:, ST[st]],
                    ktm[i][m][:, b * SENC : (b + 1) * SENC],
                    qt[:, H + i, ST[st]],
                    start=False,
                    stop=True,
                )
            exps = expp.tile([SENC, S], BF16, tag="exps", name=f"ex{b}_{h}")
            nc.scalar.activation(
                out=exps, in_=ps_s, func=AF.Exp, scale=ATTN_SCALE
            )
            return exps

        def attn_back(b, at, h, exps):
            i, m = divmod(h, 4)
            ps_av, ps_lo = [], []
            for st in range(2):
                av = psav.tile([P, 512], F32, tag="av", name=f"av{b}_{h}_{st}")
                nc.tensor.matmul(
                    av, v_nat[b][:, P * h : P * (h + 1)], exps[:, ST[st]],
                    start=True, stop=True,
                )
                ps_av.append(av)
                lo = pslo.tile([33, 512], F32, tag="lo", name=f"lo{b}_{h}_{st}")
                nc.tensor.matmul(
                    lo, vlo[b][:, h, :], exps[:, ST[st]],
                    start=True, stop=True,
                )
                ps_lo.append(lo)
            rec = recp.tile([1, S], F32, tag="rec", name=f"rec{b}_{h}")
            for st in range(2):
                nc.vector.reciprocal(
                    out=rec[:, ST[st]], in_=ps_lo[st][32:33, :]
                )
            bc = bcp.tile([P, S], F32, tag="bc", name=f"bc{b}_{h}")
            nc.gpsimd.partition_broadcast(bc, rec)
            lo = lost.tile([32, S], BF16, tag="lo", name=f"lost{b}_{h}")
            for st in range(2):
                nc.vector.scalar_tensor_tensor(
                    out=at[:, h, ST[st]], in0=ps_av[st], scalar=1.0,
                    in1=bc[:, ST[st]], op0=MULT, op1=MULT,
                )
                nc.vector.scalar_tensor_tensor(
                    out=lo[:, ST[st]], in0=ps_lo[st][0:32, :], scalar=1.0,
                    in1=bc[0:32, ST[st]], op0=MULT, op1=MULT,
                )
            nc.sync.dma_start(
                out=at[32 * m : 32 * m + 32, H + i, :], in_=lo
            )

        # ---- phase A: Q proj batch 0 ----
        qt0 = apool.tile([P, NCO, S], BF16, tag="act", name="qt0")
        for co in range(NCO):
            q_unit(0, qt0, co)

        # ---- phase B: attention b0 interleaved with Q proj b1 ----
        at0 = apool.tile([P, NCI_Q, S], BF16, tag="act", name="at0")
        qt1 = apool.tile([P, NCO, S], BF16, tag="act", name="qt1")
        qco = iter(range(NCO))
        for h in range(H):
            exps = attn_front(0, qt0, h)
            for _ in range(3 if h % 4 == 0 else 1):
                co = next(qco, None)
                if co is not None:
                    q_unit(1, qt1, co)
            attn_back(0, at0, h, exps)
        for co in qco:
            q_unit(1, qt1, co)

        # ---- phase C: attention b1 interleaved with O proj b0 ----
        at1 = apool.tile([P, NCI_Q, S], BF16, tag="act", name="at1")
        oco = iter(range(NCO))
        for h in range(H):
            exps = attn_front(1, qt1, h)
            for _ in range(2 if h % 4 == 0 else 1):
                co = next(oco, None)
                if co is not None:
                    o_unit(0, at0, co)
            attn_back(1, at1, h, exps)
        for co in oco:
            o_unit(0, at0, co)

        # ---- phase D: O proj batch 1 ----
        for co in range(NCO):
            o_unit(1, at1, co)

    nc.compile()
    return nc


_NC_CACHE = []


def _get_nc():
    if not _NC_CACHE:
        _NC_CACHE.append(build())
    return _NC_CACHE[0]


def make_in_maps(hidden_states, encoder_hidden_states, Wq, Wk, Wv, Wo,
                 q_down, q_up, k_down, k_up, v_down, v_up, o_down, o_up):
    f64 = np.float64
    wq = Wq.astype(f64) + q_up.astype(f64) @ q_down.astype(f64)
    wk = Wk.astype(f64) + k_up.astype(f64) @ k_down.astype(f64)
    wv = Wv.astype(f64) + v_up.astype(f64) @ v_down.astype(f64)
    wo = Wo.astype(f64) + o_up.astype(f64) @ o_down.astype(f64)
    perm = head_perm()
    bf = ml_dtypes.bfloat16
    # stationary layouts: w*[ci, p, c] = W_p[c, ci*128+p]  (= W_p.T reshaped)
    wq_h = np.ascontiguousarray(wq[perm, :].T.reshape(NCI_Q, P, C).astype(bf))
    wk_h = np.ascontiguousarray(wk[perm, :].T.reshape(NCI_KV, P, C).astype(bf))
    wv_h = np.ascontiguousarray(wv[perm, :].T.reshape(NCI_KV, P, C).astype(bf))
    wo_h = np.ascontiguousarray(wo[:, perm].T.reshape(NCI_Q, P, C).astype(bf))

    in_maps = []
    for c in range(NCORES):
        hs = hidden_states[c * BPC : (c + 1) * BPC]  # [2, S, C]
        xt = np.ascontiguousarray(
            hs.transpose(0, 2, 1).reshape(BPC, NCI_Q, P, S).astype(bf)
        )
        enc = encoder_hidden_states[c * BPC : (c + 1) * BPC]  # [2, 77, 1024]
        et = np.zeros((CENC, EW), np.float32)
        for b in range(BPC):
            et[:, b * SENC : (b + 1) * SENC] = enc[b].T
        et = np.ascontiguousarray(et.reshape(NCI_KV, P, EW).astype(bf))
        in_maps.append(
            {"xt": xt, "et": et, "wq": wq_h, "wk": wk_h, "wv": wv_h,
             "wo": wo_h}
        )
    return in_maps


def kernel(hidden_states, encoder_hidden_states, Wq, Wk, Wv, Wo, bo,
           q_down, q_up, k_down, k_up, v_down, v_up, o_down, o_up):
    nc = _get_nc()
    in_maps = make_in_maps(
        hidden_states, encoder_hidden_states, Wq, Wk, Wv, Wo,
        q_down, q_up, k_down, k_up, v_down, v_up, o_down, o_up,
    )
    res = run_bass_kernel_spmd(nc, in_maps, list(range(NCORES)))
    out = np.empty((B, S, C), np.float32)
    for c in range(NCORES):
        ot = res.results[c]["otd"]  # [BPC, C, S]
        for b in range(BPC):
            out[c * BPC + b] = ot[b].T
    out += bo.astype(np.float32)[None, None, :]
    return out
